# revision 12
# baseline (speedup 1.0000x reference)
"""Trainium2 Bass kernel for nn_CrossAttentionGenerator.

Pipeline (per core; 8 cores = 4 batches x 2 halves of N):
  - MLP features for the core's source half (4096 rows) and the full target
    (8192 rows) of its batch: Linear(3->64) -> LayerNorm -> ReLU -> Linear(64->64).
    Target features (WITHOUT b2 -- a per-k-constant logit shift, softmax
    invariant) + coords go to a DRAM table for gathering.
  - Distance phase per 128-row source tile: v = 2 s.t - |t|^2 (|s|^2 dropped:
    per-row constant, ranking-invariant) via split-bf16 matmul (K=21) into
    PSUM, evacuated to SBUF by the scalar engine.
  - Top-8 per row: nc.vector.max + nc.vector.max_index into persistent slabs,
    so the vector-bound selection stream never blocks on the attention chain.
  - Attention (1-tile lag behind selection): gather [feat|coords] rows via
    per-partition indirect DMA, logits = srcF . K / temp, softmax, attended.

Self-contained: hardcodes B=4, N=M=8192, F=64, K=8 and does all sharding
host-side inside kernel().
"""

import numpy as np

import concourse.bacc as bacc
import concourse.bass as bass
import concourse.tile as tile
import concourse.mybir as mybir
from concourse.bass_utils import run_bass_kernel_spmd
from concourse.masks import make_identity

import ml_dtypes

bf16 = ml_dtypes.bfloat16
f32 = mybir.dt.float32
bf16_t = mybir.dt.bfloat16
u16 = mybir.dt.uint16
u32 = mybir.dt.uint32

B, N, M, F = 4, 8192, 8192, 64
K_NN = 8
N_SH = N // 2            # rows per core
TILES = N_SH // 128      # 32 source tiles
MTILES = M // 128        # 64 target tiles
K_SPLIT = 21             # split-bf16 distance matmul contraction size
TBL_W = 68               # feat(64) + coords(3) + pad(1)
LN_EPS = 1e-5

_CACHE = {}


def _split3(x):
    """fp32 -> three bf16 planes (h+m+l reconstructs x to ~2^-24 rel)."""
    h = x.astype(bf16).astype(np.float32)
    r = (x - h).astype(np.float32)
    m = r.astype(bf16).astype(np.float32)
    l = (r - m).astype(np.float32).astype(bf16).astype(np.float32)
    return h, m, l


def _build_dist_strips(s, t):
    """lhsT (21, n) and rhs (21, m) bf16 strips for v = 2 s.t - |t|^2.

    |s|^2 is omitted: it is constant along each distance row, so the top-k
    ranking and the matched values' indices are unchanged.  Per coordinate
    the six O(>=2^-24) product terms are kept: hh hm mh mm hl lh; the |t|^2
    column constant is subtracted via three (-1, t2*) rows.
    """
    n = s.shape[0]; m = t.shape[0]
    sh, sm, sl = _split3(2.0 * s)
    th, tm_, tl = _split3(t)
    tsq = ((t[:, 0] * t[:, 0]).astype(np.float32)
           + (t[:, 1] * t[:, 1]).astype(np.float32))
    tsq = (tsq.astype(np.float32) + (t[:, 2] * t[:, 2]).astype(np.float32)).astype(np.float32)
    t2h, t2m, t2l = _split3(tsq)
    ones_n = np.ones((n,), np.float32)

    lhs_rows = []
    rhs_rows = []
    for c in range(3):
        pairs = [(sh[:, c], th[:, c]), (sh[:, c], tm_[:, c]), (sm[:, c], th[:, c]),
                 (sm[:, c], tm_[:, c]), (sh[:, c], tl[:, c]), (sl[:, c], th[:, c])]
        for a, b in pairs:
            lhs_rows.append(a)
            rhs_rows.append(b)
    for r in (t2h, t2m, t2l):
        lhs_rows.append(-ones_n)
        rhs_rows.append(r)
    lhsT = np.stack(lhs_rows).astype(bf16)   # (21, n)
    rhs = np.stack(rhs_rows).astype(bf16)    # (21, m)
    return lhsT, rhs


def _build_program(trivial_ln):
    nc = bacc.Bacc("TRN2", target_bir_lowering=False, num_devices=8)

    # ---- I/O -------------------------------------------------------------
    lhsT_d = nc.dram_tensor("lhsT", [K_SPLIT, N_SH], bf16_t, kind="ExternalInput")
    rhs_d = nc.dram_tensor("rhs", [K_SPLIT, M], bf16_t, kind="ExternalInput")
    srcT4_d = nc.dram_tensor("srcT4", [4, N_SH], f32, kind="ExternalInput")
    tgtT4_d = nc.dram_tensor("tgtT4", [4, M], f32, kind="ExternalInput")
    w1b_d = nc.dram_tensor("w1b", [4, F], f32, kind="ExternalInput")
    w2_d = nc.dram_tensor("w2", [2 * F, F], f32, kind="ExternalInput")
    b2t_d = nc.dram_tensor("b2t", [128, F], f32, kind="ExternalInput")
    ltc_d = nc.dram_tensor("ltc", [128, 1], f32, kind="ExternalInput")
    if not trivial_ln:
        lng_d = nc.dram_tensor("lng", [128, F], f32, kind="ExternalInput")
        lnb_d = nc.dram_tensor("lnb", [128, F], f32, kind="ExternalInput")
    out_d = nc.dram_tensor("out", [N_SH, 3], f32, kind="ExternalOutput")

    with tile.TileContext(nc) as tc:
        import contextlib
        ctx = contextlib.ExitStack()
        with ctx:
            const = ctx.enter_context(tc.tile_pool(name="const", bufs=1))
            mlp_sb = ctx.enter_context(tc.tile_pool(name="mlp_sb", bufs=4))
            mlp_sm = ctx.enter_context(tc.tile_pool(name="mlp_sm", bufs=8))
            mlp_sq = ctx.enter_context(tc.tile_pool(name="mlp_sq", bufs=2))
            mlp_ps = ctx.enter_context(tc.tile_pool(name="mlp_ps", bufs=2, space="PSUM"))
            dist_ps = ctx.enter_context(tc.tile_pool(name="dist_ps", bufs=2, space="PSUM"))
            row_sb = ctx.enter_context(tc.tile_pool(name="row_sb", bufs=2))
            att_sb = ctx.enter_context(tc.tile_pool(name="att_sb", bufs=3))
            dram = ctx.enter_context(tc.tile_pool(name="dram", bufs=1, space="DRAM"))

            # ---- constants / strips -------------------------------------
            lhsT = const.tile([K_SPLIT, N_SH], bf16_t)
            nc.sync.dma_start(lhsT[:], lhsT_d[:])
            rhs = const.tile([K_SPLIT, M], bf16_t)
            nc.sync.dma_start(rhs[:], rhs_d[:])
            srcT4 = const.tile([4, N_SH], f32)
            nc.sync.dma_start(srcT4[:], srcT4_d[:])
            tgtT4 = const.tile([4, M], f32)
            nc.sync.dma_start(tgtT4[:], tgtT4_d[:])
            w1b = const.tile([4, F], f32)
            nc.sync.dma_start(w1b[:], w1b_d[:])
            w2 = const.tile([2 * F, F], f32)   # W2 duplicated at partitions 0 and 64
            nc.sync.dma_start(w2[:], w2_d[:])
            b2t = const.tile([128, F], f32)
            nc.sync.dma_start(b2t[:], b2t_d[:])
            ltc = const.tile([128, 1], f32)
            nc.sync.dma_start(ltc[:], ltc_d[:])
            if not trivial_ln:
                lng = const.tile([128, F], f32)
                nc.sync.dma_start(lng[:], lng_d[:])
                lnb = const.tile([128, F], f32)
                nc.sync.dma_start(lnb[:], lnb_d[:])

            ident = const.tile([128, 128], f32)
            make_identity(nc, ident[:])

            # inv temperature column: 0.125 * exp(-log_temp)
            invt = const.tile([128, 1], f32)
            nc.scalar.activation(invt[:], ltc[:], mybir.ActivationFunctionType.Exp,
                                 scale=-1.0)
            nc.vector.tensor_scalar_mul(invt[:], invt[:], 1.0 / 8.0)

            srcF = const.tile([128, TILES * F], f32)       # source features (+b2)
            outacc = const.tile([128, TILES * 3], f32)     # attended accumulator
            v8slab = const.tile([128, TILES * 8], f32)     # top-8 values per tile
            m8slab = const.tile([128, TILES * 8], u32)     # top-8 indices per tile

            # feature table in DRAM: [feat(64) | coords(3) | pad]
            ftable = dram.tile([M, TBL_W], f32)
            # coords columns from tgtT4 rows 0:3 (one strided DMA)
            nc.sync.dma_start(
                ftable[:, 64:67].rearrange("m c -> c m"), tgtT4[0:3, :])

            # ---- MLP: batched-stats structure ---------------------------
            # Phase A: mm1 for every tile -> h slab (PSUM freed immediately).
            # Phase B: segmented bn_stats over 8 tiles per call + slab-wide
            #          istd/nmu chain  (a handful of vector ops total, so the
            #          MLP never competes with the 8.7us selection scans for
            #          vector slots).
            # Phase C: per pair: z=Relu(h*istd+nmu), one 128x128 transpose,
            #          mm2, feature writeback.
            NT = MTILES + TILES                         # 96 tiles total
            hslab = const.tile([128, NT * F], f32)      # 24KB/partition
            sums = const.tile([128, NT], f32)           # per-tile sum(h)
            sumsq = const.tile([128, NT], f32)          # per-tile sum(h^2)
            istds = const.tile([128, NT], f32)
            nmus = const.tile([128, NT], f32)

            def mlp_A(idx, xap):
                h_ps = mlp_ps.tile([128, F], f32, tag="mm")
                nc.tensor.matmul(h_ps[:], lhsT=xap, rhs=w1b[:],
                                 start=True, stop=True)
                # vector is idle in the early phase; scalar is not
                nc.vector.tensor_copy(out=hslab[:, idx * F:(idx + 1) * F],
                                      in_=h_ps[:])

            def mlp_stats(g):
                # segmented sum / sum-of-squares over 8 tiles at once
                sl = hslab[:, g * 8 * F:(g + 1) * 8 * F].rearrange(
                    "p (t f) -> p t f", f=F)
                sq = mlp_sq.tile([128, 8, F], f32, tag="sq")
                nc.scalar.activation(sq[:], sl,
                                     mybir.ActivationFunctionType.Square)
                nc.vector.tensor_reduce(
                    out=sums[:, g * 8:(g + 1) * 8], in_=sl,
                    op=mybir.AluOpType.add, axis=mybir.AxisListType.X)
                nc.vector.tensor_reduce(
                    out=sumsq[:, g * 8:(g + 1) * 8], in_=sq[:],
                    op=mybir.AluOpType.add, axis=mybir.AxisListType.X)

            def mlp_chain(lo, nk):
                # istd = 1/sqrt(E[h^2] - mu^2 + eps); nmu = -mu*istd
                mu = mlp_sm.tile([128, nk], f32, tag="mu%d" % lo)
                nc.vector.tensor_scalar_mul(mu[:], sums[:, lo:lo + nk], 1.0 / F)
                var = mlp_sm.tile([128, nk], f32, tag="var%d" % lo)
                nc.vector.scalar_tensor_tensor(
                    out=var[:], in0=mu[:], scalar=-1.0, in1=mu[:],
                    op0=mybir.AluOpType.mult, op1=mybir.AluOpType.mult)
                nc.vector.scalar_tensor_tensor(
                    out=var[:], in0=sumsq[:, lo:lo + nk], scalar=1.0 / F,
                    in1=var[:],
                    op0=mybir.AluOpType.mult, op1=mybir.AluOpType.add)
                nc.vector.tensor_scalar_add(var[:], var[:], LN_EPS)
                nc.vector.reciprocal(var[:], var[:])
                nc.scalar.activation(istds[:, lo:lo + nk], var[:],
                                     mybir.ActivationFunctionType.Sqrt)
                nc.vector.scalar_tensor_tensor(
                    out=nmus[:, lo:lo + nk], in0=mu[:],
                    scalar=-1.0, in1=istds[:, lo:lo + nk],
                    op0=mybir.AluOpType.mult, op1=mybir.AluOpType.mult)

            def mlp_C(pair, dst_kind):
                # global tile indices a, a+1
                a = 2 * pair
                z2 = mlp_sb.tile([128, 2 * F], f32, tag="z2")
                for j in range(2):
                    idx = a + j
                    z = z2[:, j * F:(j + 1) * F]
                    h = hslab[:, idx * F:(idx + 1) * F]
                    if trivial_ln:
                        nc.scalar.activation(z, h,
                                             mybir.ActivationFunctionType.Relu,
                                             bias=nmus[:, idx:idx + 1],
                                             scale=istds[:, idx:idx + 1])
                    else:
                        zn = mlp_sb.tile([128, F], f32, tag="zn")
                        nc.scalar.activation(zn[:], h,
                                             mybir.ActivationFunctionType.Identity,
                                             bias=nmus[:, idx:idx + 1],
                                             scale=istds[:, idx:idx + 1])
                        nc.vector.scalar_tensor_tensor(
                            out=zn[:], in0=zn[:], scalar=1.0, in1=lng[:],
                            op0=mybir.AluOpType.mult, op1=mybir.AluOpType.mult)
                        nc.vector.scalar_tensor_tensor(
                            out=zn[:], in0=zn[:], scalar=0.0, in1=lnb[:],
                            op0=mybir.AluOpType.add, op1=mybir.AluOpType.add)
                        nc.vector.tensor_relu(z, zn[:])
                zt_ps = mlp_ps.tile([128, 128], f32, tag="tr")
                nc.tensor.transpose(out=zt_ps[:], in_=z2[:], identity=ident[:])
                hT2 = mlp_sb.tile([128, 128], f32, tag="hT2")
                nc.scalar.copy(hT2[:], zt_ps[:])
                for j in range(2):
                    f_ps = mlp_ps.tile([128, F], f32, tag="mm")
                    nc.tensor.matmul(f_ps[:], lhsT=hT2[j * F:(j + 1) * F, :],
                                     rhs=w2[j * F:(j + 1) * F, :],
                                     start=True, stop=True)
                    if dst_kind == "tgt":
                        t_idx = a + j
                        feat = mlp_sb.tile([128, F], f32, tag="feat")
                        nc.vector.tensor_copy(out=feat[:], in_=f_ps[:])
                        nc.sync.dma_start(
                            ftable[t_idx * 128:(t_idx + 1) * 128, 0:F], feat[:])
                    else:
                        t_idx = a + j - MTILES
                        nc.vector.tensor_tensor(
                            out=srcF[:, t_idx * F:(t_idx + 1) * F],
                            in0=f_ps[:], in1=b2t[:], op=mybir.AluOpType.add)

            # target tiles first so ftable completes early
            for i in range(MTILES):
                mlp_A(i, tgtT4[:, i * 128:(i + 1) * 128])
            for g in range(MTILES // 8):
                mlp_stats(g)
            mlp_chain(0, MTILES)
            for p in range(MTILES // 2):
                mlp_C(p, "tgt")
            for i in range(TILES):
                mlp_A(MTILES + i, srcT4[:, i * 128:(i + 1) * 128])
            for g in range(MTILES // 8, NT // 8):
                mlp_stats(g)
            mlp_chain(MTILES, TILES)
            for p in range(MTILES // 2, NT // 2):
                mlp_C(p, "src")

            # ---- selection per tile (vector-bound stream) ---------------
            NGRP = 8          # PSUM evacuation groups per tile
            GW = M // NGRP    # 1024 columns per group

            def select_tile(t):
                lhs_t = lhsT[:, t * 128:(t + 1) * 128]
                row = row_sb.tile([128, M], f32, tag="row")
                for g in range(NGRP):
                    ps = dist_ps.tile([128, GW], f32, tag="d")
                    for h in range(GW // 512):
                        c0 = g * GW + h * 512
                        nc.tensor.matmul(ps[:, h * 512:(h + 1) * 512],
                                         lhsT=lhs_t,
                                         rhs=rhs[:, c0:c0 + 512],
                                         start=True, stop=True)
                    nc.scalar.copy(row[:, g * GW:(g + 1) * GW], ps[:])

                # exact top-8 (maximum v = closest) into persistent slabs
                v8 = v8slab[:, t * 8:(t + 1) * 8]
                nc.vector.max(out=v8, in_=row[:])
                nc.vector.max_index(out=m8slab[:, t * 8:(t + 1) * 8],
                                    in_max=v8, in_values=row[:])

                # gathers queued on gpsimd right away (ftable ready after MLP)
                gath = att_sb.tile([128, K_NN, TBL_W], f32, tag="gath")
                for k in range(K_NN):
                    nc.gpsimd.indirect_dma_start(
                        out=gath[:, k, :], out_offset=None,
                        in_=ftable[:],
                        in_offset=bass.IndirectOffsetOnAxis(
                            ap=m8slab[:, t * 8 + k:t * 8 + k + 1], axis=0))
                return gath

            def attend_tile(t, gath):
                # logits_k = srcF . K_k
                logits = att_sb.tile([128, K_NN], f32, tag="logits")
                scr = att_sb.tile([128, K_NN, F], f32, tag="scr")
                sfb = srcF[:, t * F:(t + 1) * F].rearrange(
                    "p (o f) -> p o f", o=1).to_broadcast([128, K_NN, F])
                nc.vector.scalar_tensor_tensor(
                    out=scr[:], in0=gath[:, :, 0:F], scalar=1.0, in1=sfb,
                    op0=mybir.AluOpType.mult, op1=mybir.AluOpType.mult)
                nc.vector.tensor_reduce(
                    out=logits[:], in_=scr[:],
                    op=mybir.AluOpType.add, axis=mybir.AxisListType.X)
                # softmax over k with temperature
                mx = att_sb.tile([128, 1], f32, tag="mx")
                nc.vector.tensor_reduce(out=mx[:], in_=logits[:],
                                        op=mybir.AluOpType.max,
                                        axis=mybir.AxisListType.X)
                bcol = att_sb.tile([128, 1], f32, tag="bcol")
                nc.vector.scalar_tensor_tensor(
                    out=bcol[:], in0=mx[:], scalar=-1.0, in1=invt[:],
                    op0=mybir.AluOpType.mult, op1=mybir.AluOpType.mult)
                att = att_sb.tile([128, K_NN], f32, tag="att")
                nc.scalar.activation(att[:], logits[:],
                                     mybir.ActivationFunctionType.Exp,
                                     bias=bcol[:], scale=invt[:])
                ssum = att_sb.tile([128, 1], f32, tag="ssum")
                nc.vector.tensor_reduce(out=ssum[:], in_=att[:],
                                        op=mybir.AluOpType.add,
                                        axis=mybir.AxisListType.X)
                rs = att_sb.tile([128, 1], f32, tag="rs")
                nc.vector.reciprocal(rs[:], ssum[:])
                nc.vector.tensor_scalar_mul(att[:], att[:], rs[:])
                # attended = sum_k att * V  (V = gathered coords)
                prod = att_sb.tile([128, K_NN, 3], f32, tag="prod")
                nc.vector.scalar_tensor_tensor(
                    out=prod[:], in0=gath[:, :, F:F + 3], scalar=1.0,
                    in1=att[:].to_broadcast([128, K_NN, 3]),
                    op0=mybir.AluOpType.mult, op1=mybir.AluOpType.mult)
                nc.vector.tensor_reduce(
                    out=outacc[:, t * 3:(t + 1) * 3],
                    in_=prod[:].rearrange("p k c -> p c k"),
                    op=mybir.AluOpType.add, axis=mybir.AxisListType.X)

            pend = None
            for t in range(TILES):
                gath = select_tile(t)
                if pend is not None:
                    attend_tile(t - 1, pend)
                pend = gath
            attend_tile(TILES - 1, pend)

            # ---- write output -------------------------------------------
            nc.sync.dma_start(
                out_d[:].rearrange("(t p) c -> p t c", p=128),
                outacc[:].rearrange("p (t c) -> p t c", c=3))

    nc.compile()
    return nc


def _get_program(trivial_ln):
    key = ("prog", trivial_ln)
    if key not in _CACHE:
        _CACHE[key] = _build_program(trivial_ln)
    return _CACHE[key]


def kernel(source, target, W1, b1, ln_g, ln_b, W2, b2, log_temp):
    source = np.ascontiguousarray(np.asarray(source, dtype=np.float32))
    target = np.ascontiguousarray(np.asarray(target, dtype=np.float32))
    W1 = np.asarray(W1, np.float32); b1 = np.asarray(b1, np.float32)
    ln_g = np.asarray(ln_g, np.float32); ln_b = np.asarray(ln_b, np.float32)
    W2 = np.asarray(W2, np.float32); b2 = np.asarray(b2, np.float32)
    log_temp = np.asarray(log_temp, np.float32)

    trivial_ln = bool(np.all(ln_g == 1.0) and np.all(ln_b == 0.0))
    nc = _get_program(trivial_ln)

    w1b = np.concatenate([W1, b1[None, :]], axis=0).astype(np.float32)       # (4, 64)
    b2t = np.tile(b2[None, :], (128, 1)).astype(np.float32)
    ltc = np.full((128, 1), float(log_temp[0]), np.float32)

    in_maps = []
    for c in range(8):
        b = c // 2
        h = c % 2
        s = source[b, h * N_SH:(h + 1) * N_SH]      # (4096, 3)
        t = target[b]                                # (8192, 3)
        lhsT, rhs = _build_dist_strips(s, t)
        srcT4 = np.concatenate([s.T, np.ones((1, N_SH), np.float32)], axis=0)
        tgtT4 = np.concatenate([t.T, np.ones((1, M), np.float32)], axis=0)
        im = {
            "lhsT": np.ascontiguousarray(lhsT),
            "rhs": np.ascontiguousarray(rhs),
            "srcT4": np.ascontiguousarray(srcT4.astype(np.float32)),
            "tgtT4": np.ascontiguousarray(tgtT4.astype(np.float32)),
            "w1b": w1b,
            "w2": np.ascontiguousarray(np.concatenate([W2, W2], axis=0)),
            "b2t": b2t,
            "ltc": ltc,
        }
        if not trivial_ln:
            im["lng"] = np.tile(ln_g[None, :], (128, 1)).astype(np.float32)
            im["lnb"] = np.tile(ln_b[None, :], (128, 1)).astype(np.float32)
        in_maps.append(im)

    global _last_in_maps
    _last_in_maps = in_maps
    res = run_bass_kernel_spmd(nc, in_maps, core_ids=list(range(8)))
    out = np.zeros((B, N, 3), np.float32)
    for c in range(8):
        b = c // 2
        h = c % 2
        out[b, h * N_SH:(h + 1) * N_SH] = res.results[c]["out"]
    return out


# revision 13
# speedup vs baseline: 1.2368x; 1.2368x over previous
"""Trainium2 Bass kernel for nn_CrossAttentionGenerator.

Pipeline (per core; 8 cores = 4 batches x 2 halves of N):
  - MLP features for the core's source half (4096 rows) and the full target
    (8192 rows) of its batch: Linear(3->64) -> LayerNorm -> ReLU -> Linear(64->64).
    Target features (WITHOUT b2 -- a per-k-constant logit shift, softmax
    invariant) + coords go to a DRAM table for gathering.
  - Distance phase per 128-row source tile: v = 2 s.t - |t|^2 (|s|^2 dropped:
    per-row constant, ranking-invariant) via split-bf16 matmul (K=21) into
    PSUM, evacuated to SBUF by the scalar engine.
  - Top-8 per row: nc.vector.max + nc.vector.max_index into persistent slabs,
    so the vector-bound selection stream never blocks on the attention chain.
  - Attention (1-tile lag behind selection): gather [feat|coords] rows via
    per-partition indirect DMA, logits = srcF . K / temp, softmax, attended.

Self-contained: hardcodes B=4, N=M=8192, F=64, K=8 and does all sharding
host-side inside kernel().
"""

import numpy as np

import concourse.bacc as bacc
import concourse.bass as bass
import concourse.tile as tile
import concourse.mybir as mybir
from concourse.bass_utils import run_bass_kernel_spmd
from concourse.masks import make_identity

import ml_dtypes

bf16 = ml_dtypes.bfloat16
f32 = mybir.dt.float32
bf16_t = mybir.dt.bfloat16
u16 = mybir.dt.uint16
u32 = mybir.dt.uint32

B, N, M, F = 4, 8192, 8192, 64
K_NN = 8
N_SH = N // 2            # rows per core
TILES = N_SH // 128      # 32 source tiles
MTILES = M // 128        # 64 target tiles
K_SPLIT = 21             # split-bf16 distance matmul contraction size
TBL_W = 68               # feat(64) + coords(3) + pad(1)
LN_EPS = 1e-5

_CACHE = {}


def _split3(x):
    """fp32 -> three bf16 planes (h+m+l reconstructs x to ~2^-24 rel)."""
    h = x.astype(bf16).astype(np.float32)
    r = (x - h).astype(np.float32)
    m = r.astype(bf16).astype(np.float32)
    l = (r - m).astype(np.float32).astype(bf16).astype(np.float32)
    return h, m, l


def _build_dist_strips(s, t):
    """lhsT (21, n) and rhs (21, m) bf16 strips for v = 2 s.t - |t|^2.

    |s|^2 is omitted: it is constant along each distance row, so the top-k
    ranking and the matched values' indices are unchanged.  Per coordinate
    the six O(>=2^-24) product terms are kept: hh hm mh mm hl lh; the |t|^2
    column constant is subtracted via three (-1, t2*) rows.
    """
    n = s.shape[0]; m = t.shape[0]
    sh, sm, sl = _split3(2.0 * s)
    th, tm_, tl = _split3(t)
    tsq = ((t[:, 0] * t[:, 0]).astype(np.float32)
           + (t[:, 1] * t[:, 1]).astype(np.float32))
    tsq = (tsq.astype(np.float32) + (t[:, 2] * t[:, 2]).astype(np.float32)).astype(np.float32)
    t2h, t2m, t2l = _split3(tsq)
    ones_n = np.ones((n,), np.float32)

    lhs_rows = []
    rhs_rows = []
    for c in range(3):
        pairs = [(sh[:, c], th[:, c]), (sh[:, c], tm_[:, c]), (sm[:, c], th[:, c]),
                 (sm[:, c], tm_[:, c]), (sh[:, c], tl[:, c]), (sl[:, c], th[:, c])]
        for a, b in pairs:
            lhs_rows.append(a)
            rhs_rows.append(b)
    for r in (t2h, t2m, t2l):
        lhs_rows.append(-ones_n)
        rhs_rows.append(r)
    lhsT = np.stack(lhs_rows).astype(bf16)   # (21, n)
    rhs = np.stack(rhs_rows).astype(bf16)    # (21, m)
    return lhsT, rhs


def _build_program(trivial_ln):
    nc = bacc.Bacc("TRN2", target_bir_lowering=False, num_devices=8)

    # ---- I/O -------------------------------------------------------------
    lhsT_d = nc.dram_tensor("lhsT", [K_SPLIT, N_SH], bf16_t, kind="ExternalInput")
    rhs_d = nc.dram_tensor("rhs", [K_SPLIT, M], bf16_t, kind="ExternalInput")
    srcT4_d = nc.dram_tensor("srcT4", [4, N_SH], f32, kind="ExternalInput")
    tgtT4_d = nc.dram_tensor("tgtT4", [4, M], f32, kind="ExternalInput")
    w1b_d = nc.dram_tensor("w1b", [4, F], f32, kind="ExternalInput")
    w2_d = nc.dram_tensor("w2", [2 * F, F], f32, kind="ExternalInput")
    b2t_d = nc.dram_tensor("b2t", [128, F], f32, kind="ExternalInput")
    ltc_d = nc.dram_tensor("ltc", [128, 1], f32, kind="ExternalInput")
    if not trivial_ln:
        lng_d = nc.dram_tensor("lng", [128, F], f32, kind="ExternalInput")
        lnb_d = nc.dram_tensor("lnb", [128, F], f32, kind="ExternalInput")
    out_d = nc.dram_tensor("out", [N_SH, 3], f32, kind="ExternalOutput")

    with tile.TileContext(nc) as tc:
        import contextlib
        ctx = contextlib.ExitStack()
        with ctx:
            const = ctx.enter_context(tc.tile_pool(name="const", bufs=1))
            mlp_sb = ctx.enter_context(tc.tile_pool(name="mlp_sb", bufs=4))
            mlp_sm = ctx.enter_context(tc.tile_pool(name="mlp_sm", bufs=8))
            mlp_sq = ctx.enter_context(tc.tile_pool(name="mlp_sq", bufs=2))
            mlp_ps = ctx.enter_context(tc.tile_pool(name="mlp_ps", bufs=2, space="PSUM"))
            dist_ps = ctx.enter_context(tc.tile_pool(name="dist_ps", bufs=2, space="PSUM"))
            row_sb = ctx.enter_context(tc.tile_pool(name="row_sb", bufs=2))
            att_sb = ctx.enter_context(tc.tile_pool(name="att_sb", bufs=3))
            dram = ctx.enter_context(tc.tile_pool(name="dram", bufs=1, space="DRAM"))

            # ---- constants / strips -------------------------------------
            lhsT = const.tile([K_SPLIT, N_SH], bf16_t)
            nc.sync.dma_start(lhsT[:], lhsT_d[:])
            rhs = const.tile([K_SPLIT, M], bf16_t)
            nc.sync.dma_start(rhs[:], rhs_d[:])
            srcT4 = const.tile([4, N_SH], f32)
            nc.sync.dma_start(srcT4[:], srcT4_d[:])
            tgtT4 = const.tile([4, M], f32)
            nc.sync.dma_start(tgtT4[:], tgtT4_d[:])
            w1b = const.tile([4, F], f32)
            nc.sync.dma_start(w1b[:], w1b_d[:])
            w2 = const.tile([2 * F, F], f32)   # W2 duplicated at partitions 0 and 64
            nc.sync.dma_start(w2[:], w2_d[:])
            b2t = const.tile([128, F], f32)
            nc.sync.dma_start(b2t[:], b2t_d[:])
            ltc = const.tile([128, 1], f32)
            nc.sync.dma_start(ltc[:], ltc_d[:])
            if not trivial_ln:
                lng = const.tile([128, F], f32)
                nc.sync.dma_start(lng[:], lng_d[:])
                lnb = const.tile([128, F], f32)
                nc.sync.dma_start(lnb[:], lnb_d[:])

            ident = const.tile([128, 128], f32)
            make_identity(nc, ident[:])

            # inv temperature column: 0.125 * exp(-log_temp)
            invt = const.tile([128, 1], f32)
            nc.scalar.activation(invt[:], ltc[:], mybir.ActivationFunctionType.Exp,
                                 scale=-1.0)
            nc.vector.tensor_scalar_mul(invt[:], invt[:], 1.0 / 8.0)

            srcF = const.tile([128, TILES * F], f32)       # source features (+b2)
            outacc = const.tile([128, TILES * 3], f32)     # attended accumulator
            v8slab = const.tile([128, TILES * 8], f32)     # top-8 values per tile
            m8slab = const.tile([128, TILES * 8], u32)     # top-8 indices per tile

            # feature table in DRAM: [feat(64) | coords(3) | pad]
            ftable = dram.tile([M, TBL_W], f32)
            # coords columns from tgtT4 rows 0:3 (one strided DMA)
            nc.sync.dma_start(
                ftable[:, 64:67].rearrange("m c -> c m"), tgtT4[0:3, :])

            # ---- MLP: batched-stats structure ---------------------------
            # Phase A: mm1 for every tile -> h slab (PSUM freed immediately).
            # Phase B: segmented bn_stats over 8 tiles per call + slab-wide
            #          istd/nmu chain  (a handful of vector ops total, so the
            #          MLP never competes with the 8.7us selection scans for
            #          vector slots).
            # Phase C: per pair: z=Relu(h*istd+nmu), one 128x128 transpose,
            #          mm2, feature writeback.
            NT = MTILES + TILES                         # 96 tiles total
            hslab = const.tile([128, NT * F], f32)      # 24KB/partition
            sums = const.tile([128, NT], f32)           # per-tile sum(h)
            sumsq = const.tile([128, NT], f32)          # per-tile sum(h^2)
            istds = const.tile([128, NT], f32)
            nmus = const.tile([128, NT], f32)

            def mlp_A(idx, xap):
                h_ps = mlp_ps.tile([128, F], f32, tag="mm")
                nc.tensor.matmul(h_ps[:], lhsT=xap, rhs=w1b[:],
                                 start=True, stop=True)
                nc.scalar.copy(hslab[:, idx * F:(idx + 1) * F], h_ps[:])

            def mlp_stats(g):
                # segmented sum / sum-of-squares over 8 tiles at once
                sl = hslab[:, g * 8 * F:(g + 1) * 8 * F].rearrange(
                    "p (t f) -> p t f", f=F)
                sq = mlp_sq.tile([128, 8, F], f32, tag="sq")
                nc.scalar.activation(sq[:], sl,
                                     mybir.ActivationFunctionType.Square)
                nc.vector.tensor_reduce(
                    out=sums[:, g * 8:(g + 1) * 8], in_=sl,
                    op=mybir.AluOpType.add, axis=mybir.AxisListType.X)
                nc.vector.tensor_reduce(
                    out=sumsq[:, g * 8:(g + 1) * 8], in_=sq[:],
                    op=mybir.AluOpType.add, axis=mybir.AxisListType.X)

            def mlp_chain(lo, nk):
                # istd = 1/sqrt(E[h^2] - mu^2 + eps); nmu = -mu*istd
                mu = mlp_sm.tile([128, nk], f32, tag="mu%d" % lo)
                nc.vector.tensor_scalar_mul(mu[:], sums[:, lo:lo + nk], 1.0 / F)
                var = mlp_sm.tile([128, nk], f32, tag="var%d" % lo)
                nc.vector.scalar_tensor_tensor(
                    out=var[:], in0=mu[:], scalar=-1.0, in1=mu[:],
                    op0=mybir.AluOpType.mult, op1=mybir.AluOpType.mult)
                nc.vector.scalar_tensor_tensor(
                    out=var[:], in0=sumsq[:, lo:lo + nk], scalar=1.0 / F,
                    in1=var[:],
                    op0=mybir.AluOpType.mult, op1=mybir.AluOpType.add)
                nc.vector.tensor_scalar_add(var[:], var[:], LN_EPS)
                nc.vector.reciprocal(var[:], var[:])
                nc.scalar.activation(istds[:, lo:lo + nk], var[:],
                                     mybir.ActivationFunctionType.Sqrt)
                nc.vector.scalar_tensor_tensor(
                    out=nmus[:, lo:lo + nk], in0=mu[:],
                    scalar=-1.0, in1=istds[:, lo:lo + nk],
                    op0=mybir.AluOpType.mult, op1=mybir.AluOpType.mult)

            def mlp_C(pair, dst_kind):
                # global tile indices a, a+1
                a = 2 * pair
                z2 = mlp_sb.tile([128, 2 * F], f32, tag="z2")
                for j in range(2):
                    idx = a + j
                    z = z2[:, j * F:(j + 1) * F]
                    h = hslab[:, idx * F:(idx + 1) * F]
                    if trivial_ln:
                        nc.scalar.activation(z, h,
                                             mybir.ActivationFunctionType.Relu,
                                             bias=nmus[:, idx:idx + 1],
                                             scale=istds[:, idx:idx + 1])
                    else:
                        zn = mlp_sb.tile([128, F], f32, tag="zn")
                        nc.scalar.activation(zn[:], h,
                                             mybir.ActivationFunctionType.Identity,
                                             bias=nmus[:, idx:idx + 1],
                                             scale=istds[:, idx:idx + 1])
                        nc.vector.scalar_tensor_tensor(
                            out=zn[:], in0=zn[:], scalar=1.0, in1=lng[:],
                            op0=mybir.AluOpType.mult, op1=mybir.AluOpType.mult)
                        nc.vector.scalar_tensor_tensor(
                            out=zn[:], in0=zn[:], scalar=0.0, in1=lnb[:],
                            op0=mybir.AluOpType.add, op1=mybir.AluOpType.add)
                        nc.vector.tensor_relu(z, zn[:])
                zt_ps = mlp_ps.tile([128, 128], f32, tag="tr")
                nc.tensor.transpose(out=zt_ps[:], in_=z2[:], identity=ident[:])
                hT2 = mlp_sb.tile([128, 128], f32, tag="hT2")
                nc.scalar.copy(hT2[:], zt_ps[:])
                for j in range(2):
                    f_ps = mlp_ps.tile([128, F], f32, tag="mm")
                    nc.tensor.matmul(f_ps[:], lhsT=hT2[j * F:(j + 1) * F, :],
                                     rhs=w2[j * F:(j + 1) * F, :],
                                     start=True, stop=True)
                    if dst_kind == "tgt":
                        t_idx = a + j
                        feat = mlp_sb.tile([128, F], f32, tag="feat")
                        nc.scalar.copy(feat[:], f_ps[:])
                        nc.sync.dma_start(
                            ftable[t_idx * 128:(t_idx + 1) * 128, 0:F], feat[:])
                    else:
                        t_idx = a + j - MTILES
                        nc.vector.tensor_tensor(
                            out=srcF[:, t_idx * F:(t_idx + 1) * F],
                            in0=f_ps[:], in1=b2t[:], op=mybir.AluOpType.add)

            # target tiles first so ftable completes early
            for i in range(MTILES):
                mlp_A(i, tgtT4[:, i * 128:(i + 1) * 128])
            for g in range(MTILES // 8):
                mlp_stats(g)
            mlp_chain(0, MTILES)
            for p in range(MTILES // 2):
                mlp_C(p, "tgt")
            for i in range(TILES):
                mlp_A(MTILES + i, srcT4[:, i * 128:(i + 1) * 128])
            for g in range(MTILES // 8, NT // 8):
                mlp_stats(g)
            mlp_chain(MTILES, TILES)
            for p in range(MTILES // 2, NT // 2):
                mlp_C(p, "src")

            # ---- selection per tile (vector-bound stream) ---------------
            NGRP = 8          # PSUM evacuation groups per tile
            GW = M // NGRP    # 1024 columns per group

            def select_tile(t):
                lhs_t = lhsT[:, t * 128:(t + 1) * 128]
                row = row_sb.tile([128, M], f32, tag="row")
                for g in range(NGRP):
                    ps = dist_ps.tile([128, GW], f32, tag="d")
                    for h in range(GW // 512):
                        c0 = g * GW + h * 512
                        nc.tensor.matmul(ps[:, h * 512:(h + 1) * 512],
                                         lhsT=lhs_t,
                                         rhs=rhs[:, c0:c0 + 512],
                                         start=True, stop=True)
                    nc.scalar.copy(row[:, g * GW:(g + 1) * GW], ps[:])

                # exact top-8 (maximum v = closest) into persistent slabs
                v8 = v8slab[:, t * 8:(t + 1) * 8]
                nc.vector.max(out=v8, in_=row[:])
                nc.vector.max_index(out=m8slab[:, t * 8:(t + 1) * 8],
                                    in_max=v8, in_values=row[:])

                # gathers queued on gpsimd right away (ftable ready after MLP)
                gath = att_sb.tile([128, K_NN, TBL_W], f32, tag="gath")
                for k in range(K_NN):
                    nc.gpsimd.indirect_dma_start(
                        out=gath[:, k, :], out_offset=None,
                        in_=ftable[:],
                        in_offset=bass.IndirectOffsetOnAxis(
                            ap=m8slab[:, t * 8 + k:t * 8 + k + 1], axis=0))
                return gath

            def attend_tile(t, gath):
                # logits_k = srcF . K_k
                logits = att_sb.tile([128, K_NN], f32, tag="logits")
                scr = att_sb.tile([128, K_NN, F], f32, tag="scr")
                sfb = srcF[:, t * F:(t + 1) * F].rearrange(
                    "p (o f) -> p o f", o=1).to_broadcast([128, K_NN, F])
                nc.vector.scalar_tensor_tensor(
                    out=scr[:], in0=gath[:, :, 0:F], scalar=1.0, in1=sfb,
                    op0=mybir.AluOpType.mult, op1=mybir.AluOpType.mult)
                nc.vector.tensor_reduce(
                    out=logits[:], in_=scr[:],
                    op=mybir.AluOpType.add, axis=mybir.AxisListType.X)
                # softmax over k with temperature
                mx = att_sb.tile([128, 1], f32, tag="mx")
                nc.vector.tensor_reduce(out=mx[:], in_=logits[:],
                                        op=mybir.AluOpType.max,
                                        axis=mybir.AxisListType.X)
                bcol = att_sb.tile([128, 1], f32, tag="bcol")
                nc.vector.scalar_tensor_tensor(
                    out=bcol[:], in0=mx[:], scalar=-1.0, in1=invt[:],
                    op0=mybir.AluOpType.mult, op1=mybir.AluOpType.mult)
                att = att_sb.tile([128, K_NN], f32, tag="att")
                nc.scalar.activation(att[:], logits[:],
                                     mybir.ActivationFunctionType.Exp,
                                     bias=bcol[:], scale=invt[:])
                ssum = att_sb.tile([128, 1], f32, tag="ssum")
                nc.vector.tensor_reduce(out=ssum[:], in_=att[:],
                                        op=mybir.AluOpType.add,
                                        axis=mybir.AxisListType.X)
                rs = att_sb.tile([128, 1], f32, tag="rs")
                nc.vector.reciprocal(rs[:], ssum[:])
                nc.vector.tensor_scalar_mul(att[:], att[:], rs[:])
                # attended = sum_k att * V  (V = gathered coords)
                prod = att_sb.tile([128, K_NN, 3], f32, tag="prod")
                nc.vector.scalar_tensor_tensor(
                    out=prod[:], in0=gath[:, :, F:F + 3], scalar=1.0,
                    in1=att[:].to_broadcast([128, K_NN, 3]),
                    op0=mybir.AluOpType.mult, op1=mybir.AluOpType.mult)
                nc.vector.tensor_reduce(
                    out=outacc[:, t * 3:(t + 1) * 3],
                    in_=prod[:].rearrange("p k c -> p c k"),
                    op=mybir.AluOpType.add, axis=mybir.AxisListType.X)

            pend = None
            for t in range(TILES):
                gath = select_tile(t)
                if pend is not None:
                    attend_tile(t - 1, pend)
                pend = gath
            attend_tile(TILES - 1, pend)

            # ---- write output -------------------------------------------
            nc.sync.dma_start(
                out_d[:].rearrange("(t p) c -> p t c", p=128),
                outacc[:].rearrange("p (t c) -> p t c", c=3))

    nc.compile()
    return nc


def _get_program(trivial_ln):
    key = ("prog", trivial_ln)
    if key not in _CACHE:
        _CACHE[key] = _build_program(trivial_ln)
    return _CACHE[key]


def kernel(source, target, W1, b1, ln_g, ln_b, W2, b2, log_temp):
    source = np.ascontiguousarray(np.asarray(source, dtype=np.float32))
    target = np.ascontiguousarray(np.asarray(target, dtype=np.float32))
    W1 = np.asarray(W1, np.float32); b1 = np.asarray(b1, np.float32)
    ln_g = np.asarray(ln_g, np.float32); ln_b = np.asarray(ln_b, np.float32)
    W2 = np.asarray(W2, np.float32); b2 = np.asarray(b2, np.float32)
    log_temp = np.asarray(log_temp, np.float32)

    trivial_ln = bool(np.all(ln_g == 1.0) and np.all(ln_b == 0.0))
    nc = _get_program(trivial_ln)

    w1b = np.concatenate([W1, b1[None, :]], axis=0).astype(np.float32)       # (4, 64)
    b2t = np.tile(b2[None, :], (128, 1)).astype(np.float32)
    ltc = np.full((128, 1), float(log_temp[0]), np.float32)

    in_maps = []
    for c in range(8):
        b = c // 2
        h = c % 2
        s = source[b, h * N_SH:(h + 1) * N_SH]      # (4096, 3)
        t = target[b]                                # (8192, 3)
        lhsT, rhs = _build_dist_strips(s, t)
        srcT4 = np.concatenate([s.T, np.ones((1, N_SH), np.float32)], axis=0)
        tgtT4 = np.concatenate([t.T, np.ones((1, M), np.float32)], axis=0)
        im = {
            "lhsT": np.ascontiguousarray(lhsT),
            "rhs": np.ascontiguousarray(rhs),
            "srcT4": np.ascontiguousarray(srcT4.astype(np.float32)),
            "tgtT4": np.ascontiguousarray(tgtT4.astype(np.float32)),
            "w1b": w1b,
            "w2": np.ascontiguousarray(np.concatenate([W2, W2], axis=0)),
            "b2t": b2t,
            "ltc": ltc,
        }
        if not trivial_ln:
            im["lng"] = np.tile(ln_g[None, :], (128, 1)).astype(np.float32)
            im["lnb"] = np.tile(ln_b[None, :], (128, 1)).astype(np.float32)
        in_maps.append(im)

    global _last_in_maps
    _last_in_maps = in_maps
    res = run_bass_kernel_spmd(nc, in_maps, core_ids=list(range(8)))
    out = np.zeros((B, N, 3), np.float32)
    for c in range(8):
        b = c // 2
        h = c % 2
        out[b, h * N_SH:(h + 1) * N_SH] = res.results[c]["out"]
    return out


# revision 14
# speedup vs baseline: 1.3106x; 1.0597x over previous
"""Trainium2 Bass kernel for nn_CrossAttentionGenerator.

Pipeline (per core; 8 cores = 4 batches x 2 halves of N):
  - MLP features for the core's source half (4096 rows) and the full target
    (8192 rows) of its batch: Linear(3->64) -> LayerNorm -> ReLU -> Linear(64->64).
    Target features (WITHOUT b2 -- a per-k-constant logit shift, softmax
    invariant) + coords go to a DRAM table for gathering.
  - Distance phase per 128-row source tile: v = 2 s.t - |t|^2 (|s|^2 dropped:
    per-row constant, ranking-invariant) via split-bf16 matmul (K=21) into
    PSUM, evacuated to SBUF by the scalar engine.
  - Top-8 per row: nc.vector.max + nc.vector.max_index into persistent slabs,
    so the vector-bound selection stream never blocks on the attention chain.
  - Attention (1-tile lag behind selection): gather [feat|coords] rows via
    per-partition indirect DMA, logits = srcF . K / temp, softmax, attended.

Self-contained: hardcodes B=4, N=M=8192, F=64, K=8 and does all sharding
host-side inside kernel().
"""

import numpy as np

import concourse.bacc as bacc
import concourse.bass as bass
import concourse.tile as tile
import concourse.mybir as mybir
from concourse.bass_utils import run_bass_kernel_spmd
from concourse.masks import make_identity

import ml_dtypes

bf16 = ml_dtypes.bfloat16
f32 = mybir.dt.float32
bf16_t = mybir.dt.bfloat16
u16 = mybir.dt.uint16
u32 = mybir.dt.uint32

B, N, M, F = 4, 8192, 8192, 64
K_NN = 8
N_SH = N // 2            # rows per core
TILES = N_SH // 128      # 32 source tiles
MTILES = M // 128        # 64 target tiles
K_SPLIT = 21             # split-bf16 distance matmul contraction size
TBL_W = 68               # feat(64) + coords(3) + pad(1)
LN_EPS = 1e-5

_CACHE = {}


def _split3(x):
    """fp32 -> three bf16 planes (h+m+l reconstructs x to ~2^-24 rel)."""
    h = x.astype(bf16).astype(np.float32)
    r = (x - h).astype(np.float32)
    m = r.astype(bf16).astype(np.float32)
    l = (r - m).astype(np.float32).astype(bf16).astype(np.float32)
    return h, m, l


def _build_dist_strips(s, t):
    """lhsT (21, n) and rhs (21, m) bf16 strips for v = 2 s.t - |t|^2.

    |s|^2 is omitted: it is constant along each distance row, so the top-k
    ranking and the matched values' indices are unchanged.  Per coordinate
    the six O(>=2^-24) product terms are kept: hh hm mh mm hl lh; the |t|^2
    column constant is subtracted via three (-1, t2*) rows.
    """
    n = s.shape[0]; m = t.shape[0]
    sh, sm, sl = _split3(2.0 * s)
    th, tm_, tl = _split3(t)
    tsq = ((t[:, 0] * t[:, 0]).astype(np.float32)
           + (t[:, 1] * t[:, 1]).astype(np.float32))
    tsq = (tsq.astype(np.float32) + (t[:, 2] * t[:, 2]).astype(np.float32)).astype(np.float32)
    t2h, t2m, t2l = _split3(tsq)
    ones_n = np.ones((n,), np.float32)

    lhs_rows = []
    rhs_rows = []
    for c in range(3):
        pairs = [(sh[:, c], th[:, c]), (sh[:, c], tm_[:, c]), (sm[:, c], th[:, c]),
                 (sm[:, c], tm_[:, c]), (sh[:, c], tl[:, c]), (sl[:, c], th[:, c])]
        for a, b in pairs:
            lhs_rows.append(a)
            rhs_rows.append(b)
    for r in (t2h, t2m, t2l):
        lhs_rows.append(-ones_n)
        rhs_rows.append(r)
    lhsT = np.stack(lhs_rows).astype(bf16)   # (21, n)
    rhs = np.stack(rhs_rows).astype(bf16)    # (21, m)
    return lhsT, rhs


def _build_program(trivial_ln):
    nc = bacc.Bacc("TRN2", target_bir_lowering=False, num_devices=8)

    # ---- I/O -------------------------------------------------------------
    lhsT_d = nc.dram_tensor("lhsT", [K_SPLIT, N_SH], bf16_t, kind="ExternalInput")
    rhs_d = nc.dram_tensor("rhs", [K_SPLIT, M], bf16_t, kind="ExternalInput")
    srcT4_d = nc.dram_tensor("srcT4", [4, N_SH], f32, kind="ExternalInput")
    tgtT4_d = nc.dram_tensor("tgtT4", [4, M], f32, kind="ExternalInput")
    w1b_d = nc.dram_tensor("w1b", [4, F], f32, kind="ExternalInput")
    w2_d = nc.dram_tensor("w2", [2 * F, F], f32, kind="ExternalInput")
    b2t_d = nc.dram_tensor("b2t", [128, F], f32, kind="ExternalInput")
    ltc_d = nc.dram_tensor("ltc", [128, 1], f32, kind="ExternalInput")
    if not trivial_ln:
        lng_d = nc.dram_tensor("lng", [128, F], f32, kind="ExternalInput")
        lnb_d = nc.dram_tensor("lnb", [128, F], f32, kind="ExternalInput")
    out_d = nc.dram_tensor("out", [N_SH, 3], f32, kind="ExternalOutput")

    with tile.TileContext(nc) as tc:
        import contextlib
        ctx = contextlib.ExitStack()
        with ctx:
            const = ctx.enter_context(tc.tile_pool(name="const", bufs=1))
            mlp_sb = ctx.enter_context(tc.tile_pool(name="mlp_sb", bufs=4))
            mlp_sm = ctx.enter_context(tc.tile_pool(name="mlp_sm", bufs=8))
            mlp_sq = ctx.enter_context(tc.tile_pool(name="mlp_sq", bufs=2))
            mlp_ps = ctx.enter_context(tc.tile_pool(name="mlp_ps", bufs=2, space="PSUM"))
            dist_ps = ctx.enter_context(tc.tile_pool(name="dist_ps", bufs=2, space="PSUM"))
            row_sb = ctx.enter_context(tc.tile_pool(name="row_sb", bufs=2))
            att_sb = ctx.enter_context(tc.tile_pool(name="att_sb", bufs=3))
            dram = ctx.enter_context(tc.tile_pool(name="dram", bufs=1, space="DRAM"))

            # ---- constants / strips -------------------------------------
            lhsT = const.tile([K_SPLIT, N_SH], bf16_t)
            nc.sync.dma_start(lhsT[:], lhsT_d[:])
            rhs = const.tile([K_SPLIT, M], bf16_t)
            nc.sync.dma_start(rhs[:], rhs_d[:])
            srcT4 = const.tile([4, N_SH], f32)
            nc.sync.dma_start(srcT4[:], srcT4_d[:])
            tgtT4 = const.tile([4, M], f32)
            nc.sync.dma_start(tgtT4[:], tgtT4_d[:])
            w1b = const.tile([4, F], f32)
            nc.sync.dma_start(w1b[:], w1b_d[:])
            w2 = const.tile([2 * F, F], f32)   # W2 duplicated at partitions 0 and 64
            nc.sync.dma_start(w2[:], w2_d[:])
            b2t = const.tile([128, F], f32)
            nc.sync.dma_start(b2t[:], b2t_d[:])
            ltc = const.tile([128, 1], f32)
            nc.sync.dma_start(ltc[:], ltc_d[:])
            if not trivial_ln:
                lng = const.tile([128, F], f32)
                nc.sync.dma_start(lng[:], lng_d[:])
                lnb = const.tile([128, F], f32)
                nc.sync.dma_start(lnb[:], lnb_d[:])

            ident = const.tile([128, 128], f32)
            make_identity(nc, ident[:])

            # inv temperature column: 0.125 * exp(-log_temp)
            invt = const.tile([128, 1], f32)
            nc.scalar.activation(invt[:], ltc[:], mybir.ActivationFunctionType.Exp,
                                 scale=-1.0)
            nc.vector.tensor_scalar_mul(invt[:], invt[:], 1.0 / 8.0)

            srcF = const.tile([128, TILES * F], f32)       # source features (+b2)
            outacc = const.tile([128, TILES * 3], f32)     # attended accumulator
            v8slab = const.tile([128, TILES * 8], f32)     # top-8 values per tile
            m8slab = const.tile([128, TILES * 8], u32)     # top-8 indices per tile

            # feature table in DRAM: [feat(64) | coords(3) | pad]
            ftable = dram.tile([M, TBL_W], f32)
            # coords columns from tgtT4 rows 0:3 (one strided DMA)
            nc.sync.dma_start(
                ftable[:, 64:67].rearrange("m c -> c m"), tgtT4[0:3, :])

            # ---- MLP: batched-stats structure ---------------------------
            # Phase A: mm1 for every tile -> h slab (PSUM freed immediately).
            # Phase B: segmented bn_stats over 8 tiles per call + slab-wide
            #          istd/nmu chain  (a handful of vector ops total, so the
            #          MLP never competes with the 8.7us selection scans for
            #          vector slots).
            # Phase C: per pair: z=Relu(h*istd+nmu), one 128x128 transpose,
            #          mm2, feature writeback.
            NT = MTILES + TILES                         # 96 tiles total
            hslab = const.tile([128, NT * F], f32)      # 24KB/partition
            sums = const.tile([128, NT], f32)           # per-tile sum(h)
            sumsq = const.tile([128, NT], f32)          # per-tile sum(h^2)
            istds = const.tile([128, NT], f32)
            nmus = const.tile([128, NT], f32)

            def mlp_A(idx, xap):
                h_ps = mlp_ps.tile([128, F], f32, tag="mm")
                nc.tensor.matmul(h_ps[:], lhsT=xap, rhs=w1b[:],
                                 start=True, stop=True)
                nc.scalar.copy(hslab[:, idx * F:(idx + 1) * F], h_ps[:])

            def mlp_stats(g):
                # segmented sum / sum-of-squares over 8 tiles at once
                sl = hslab[:, g * 8 * F:(g + 1) * 8 * F].rearrange(
                    "p (t f) -> p t f", f=F)
                sq = mlp_sq.tile([128, 8, F], f32, tag="sq")
                nc.scalar.activation(sq[:], sl,
                                     mybir.ActivationFunctionType.Square)
                nc.vector.tensor_reduce(
                    out=sums[:, g * 8:(g + 1) * 8], in_=sl,
                    op=mybir.AluOpType.add, axis=mybir.AxisListType.X)
                nc.vector.tensor_reduce(
                    out=sumsq[:, g * 8:(g + 1) * 8], in_=sq[:],
                    op=mybir.AluOpType.add, axis=mybir.AxisListType.X)

            def mlp_chain(lo, nk):
                # istd = 1/sqrt(E[h^2] - mu^2 + eps); nmu = -mu*istd
                mu = mlp_sm.tile([128, nk], f32, tag="mu%d" % lo)
                nc.vector.tensor_scalar_mul(mu[:], sums[:, lo:lo + nk], 1.0 / F)
                var = mlp_sm.tile([128, nk], f32, tag="var%d" % lo)
                nc.vector.scalar_tensor_tensor(
                    out=var[:], in0=mu[:], scalar=-1.0, in1=mu[:],
                    op0=mybir.AluOpType.mult, op1=mybir.AluOpType.mult)
                nc.vector.scalar_tensor_tensor(
                    out=var[:], in0=sumsq[:, lo:lo + nk], scalar=1.0 / F,
                    in1=var[:],
                    op0=mybir.AluOpType.mult, op1=mybir.AluOpType.add)
                nc.vector.tensor_scalar_add(var[:], var[:], LN_EPS)
                nc.vector.reciprocal(var[:], var[:])
                nc.scalar.activation(istds[:, lo:lo + nk], var[:],
                                     mybir.ActivationFunctionType.Sqrt)
                nc.vector.scalar_tensor_tensor(
                    out=nmus[:, lo:lo + nk], in0=mu[:],
                    scalar=-1.0, in1=istds[:, lo:lo + nk],
                    op0=mybir.AluOpType.mult, op1=mybir.AluOpType.mult)

            def mlp_C(pair, dst_kind):
                # global tile indices a, a+1
                a = 2 * pair
                z2 = mlp_sb.tile([128, 2 * F], f32, tag="z2")
                for j in range(2):
                    idx = a + j
                    z = z2[:, j * F:(j + 1) * F]
                    h = hslab[:, idx * F:(idx + 1) * F]
                    if trivial_ln:
                        nc.scalar.activation(z, h,
                                             mybir.ActivationFunctionType.Relu,
                                             bias=nmus[:, idx:idx + 1],
                                             scale=istds[:, idx:idx + 1])
                    else:
                        zn = mlp_sb.tile([128, F], f32, tag="zn")
                        nc.scalar.activation(zn[:], h,
                                             mybir.ActivationFunctionType.Identity,
                                             bias=nmus[:, idx:idx + 1],
                                             scale=istds[:, idx:idx + 1])
                        nc.vector.scalar_tensor_tensor(
                            out=zn[:], in0=zn[:], scalar=1.0, in1=lng[:],
                            op0=mybir.AluOpType.mult, op1=mybir.AluOpType.mult)
                        nc.vector.scalar_tensor_tensor(
                            out=zn[:], in0=zn[:], scalar=0.0, in1=lnb[:],
                            op0=mybir.AluOpType.add, op1=mybir.AluOpType.add)
                        nc.vector.tensor_relu(z, zn[:])
                zt_ps = mlp_ps.tile([128, 128], f32, tag="tr")
                nc.tensor.transpose(out=zt_ps[:], in_=z2[:], identity=ident[:])
                hT2 = mlp_sb.tile([128, 128], f32, tag="hT2")
                nc.scalar.copy(hT2[:], zt_ps[:])
                for j in range(2):
                    f_ps = mlp_ps.tile([128, F], f32, tag="mm")
                    nc.tensor.matmul(f_ps[:], lhsT=hT2[j * F:(j + 1) * F, :],
                                     rhs=w2[j * F:(j + 1) * F, :],
                                     start=True, stop=True)
                    if dst_kind == "tgt":
                        t_idx = a + j
                        feat = mlp_sb.tile([128, F], f32, tag="feat")
                        nc.scalar.copy(feat[:], f_ps[:])
                        nc.sync.dma_start(
                            ftable[t_idx * 128:(t_idx + 1) * 128, 0:F], feat[:])
                    else:
                        t_idx = a + j - MTILES
                        nc.vector.tensor_tensor(
                            out=srcF[:, t_idx * F:(t_idx + 1) * F],
                            in0=f_ps[:], in1=b2t[:], op=mybir.AluOpType.add)

            # target tiles first so ftable completes early
            for i in range(MTILES):
                mlp_A(i, tgtT4[:, i * 128:(i + 1) * 128])
            for g in range(MTILES // 8):
                mlp_stats(g)
            mlp_chain(0, MTILES)
            for p in range(MTILES // 2):
                mlp_C(p, "tgt")
            for i in range(TILES):
                mlp_A(MTILES + i, srcT4[:, i * 128:(i + 1) * 128])
            for g in range(MTILES // 8, NT // 8):
                mlp_stats(g)
            mlp_chain(MTILES, TILES)
            for p in range(MTILES // 2, NT // 2):
                mlp_C(p, "src")

            # ---- selection per tile (vector-bound stream) ---------------
            NGRP = 8          # PSUM evacuation groups per tile
            GW = M // NGRP    # 1024 columns per group

            def select_tile(t):
                lhs_t = lhsT[:, t * 128:(t + 1) * 128]
                row = row_sb.tile([128, M], f32, tag="row")
                for g in range(NGRP):
                    ps = dist_ps.tile([128, GW], f32, tag="d")
                    for h in range(GW // 512):
                        c0 = g * GW + h * 512
                        nc.tensor.matmul(ps[:, h * 512:(h + 1) * 512],
                                         lhsT=lhs_t,
                                         rhs=rhs[:, c0:c0 + 512],
                                         start=True, stop=True)
                    nc.scalar.copy(row[:, g * GW:(g + 1) * GW], ps[:])

                # exact top-8 (maximum v = closest) into persistent slabs
                v8 = v8slab[:, t * 8:(t + 1) * 8]
                nc.vector.max(out=v8, in_=row[:])
                nc.vector.max_index(out=m8slab[:, t * 8:(t + 1) * 8],
                                    in_max=v8, in_values=row[:])

                # gathers queued on gpsimd right away (ftable ready after MLP)
                gath = att_sb.tile([128, K_NN, TBL_W], f32, tag="gath")
                for k in range(K_NN):
                    nc.gpsimd.indirect_dma_start(
                        out=gath[:, k, :], out_offset=None,
                        in_=ftable[:],
                        in_offset=bass.IndirectOffsetOnAxis(
                            ap=m8slab[:, t * 8 + k:t * 8 + k + 1], axis=0))
                return gath

            def attend_tile(t, gath):
                # logits_k = srcF . K_k
                logits = att_sb.tile([128, K_NN], f32, tag="logits")
                scr = att_sb.tile([128, F], f32, tag="scr")
                sf = srcF[:, t * F:(t + 1) * F]
                for k in range(K_NN):
                    nc.vector.scalar_tensor_tensor(
                        out=scr[:], in0=gath[:, k, 0:F], scalar=1.0, in1=sf,
                        op0=mybir.AluOpType.mult, op1=mybir.AluOpType.mult,
                        accum_out=logits[:, k:k + 1])
                # softmax over k with temperature
                mx = att_sb.tile([128, 1], f32, tag="mx")
                nc.vector.tensor_reduce(out=mx[:], in_=logits[:],
                                        op=mybir.AluOpType.max,
                                        axis=mybir.AxisListType.X)
                bcol = att_sb.tile([128, 1], f32, tag="bcol")
                nc.vector.scalar_tensor_tensor(
                    out=bcol[:], in0=mx[:], scalar=-1.0, in1=invt[:],
                    op0=mybir.AluOpType.mult, op1=mybir.AluOpType.mult)
                att = att_sb.tile([128, K_NN], f32, tag="att")
                nc.scalar.activation(att[:], logits[:],
                                     mybir.ActivationFunctionType.Exp,
                                     bias=bcol[:], scale=invt[:])
                ssum = att_sb.tile([128, 1], f32, tag="ssum")
                nc.vector.tensor_reduce(out=ssum[:], in_=att[:],
                                        op=mybir.AluOpType.add,
                                        axis=mybir.AxisListType.X)
                rs = att_sb.tile([128, 1], f32, tag="rs")
                nc.vector.reciprocal(rs[:], ssum[:])
                nc.vector.tensor_scalar_mul(att[:], att[:], rs[:])
                # attended = sum_k att * V  (V = gathered coords)
                prod = att_sb.tile([128, K_NN, 3], f32, tag="prod")
                nc.vector.scalar_tensor_tensor(
                    out=prod[:], in0=gath[:, :, F:F + 3], scalar=1.0,
                    in1=att[:].to_broadcast([128, K_NN, 3]),
                    op0=mybir.AluOpType.mult, op1=mybir.AluOpType.mult)
                nc.vector.tensor_reduce(
                    out=outacc[:, t * 3:(t + 1) * 3],
                    in_=prod[:].rearrange("p k c -> p c k"),
                    op=mybir.AluOpType.add, axis=mybir.AxisListType.X)

            pend = None
            for t in range(TILES):
                gath = select_tile(t)
                if pend is not None:
                    attend_tile(t - 1, pend)
                pend = gath
            attend_tile(TILES - 1, pend)

            # ---- write output -------------------------------------------
            nc.sync.dma_start(
                out_d[:].rearrange("(t p) c -> p t c", p=128),
                outacc[:].rearrange("p (t c) -> p t c", c=3))

    nc.compile()
    return nc


def _get_program(trivial_ln):
    key = ("prog", trivial_ln)
    if key not in _CACHE:
        _CACHE[key] = _build_program(trivial_ln)
    return _CACHE[key]


def kernel(source, target, W1, b1, ln_g, ln_b, W2, b2, log_temp):
    source = np.ascontiguousarray(np.asarray(source, dtype=np.float32))
    target = np.ascontiguousarray(np.asarray(target, dtype=np.float32))
    W1 = np.asarray(W1, np.float32); b1 = np.asarray(b1, np.float32)
    ln_g = np.asarray(ln_g, np.float32); ln_b = np.asarray(ln_b, np.float32)
    W2 = np.asarray(W2, np.float32); b2 = np.asarray(b2, np.float32)
    log_temp = np.asarray(log_temp, np.float32)

    trivial_ln = bool(np.all(ln_g == 1.0) and np.all(ln_b == 0.0))
    nc = _get_program(trivial_ln)

    w1b = np.concatenate([W1, b1[None, :]], axis=0).astype(np.float32)       # (4, 64)
    b2t = np.tile(b2[None, :], (128, 1)).astype(np.float32)
    ltc = np.full((128, 1), float(log_temp[0]), np.float32)

    in_maps = []
    for c in range(8):
        b = c // 2
        h = c % 2
        s = source[b, h * N_SH:(h + 1) * N_SH]      # (4096, 3)
        t = target[b]                                # (8192, 3)
        lhsT, rhs = _build_dist_strips(s, t)
        srcT4 = np.concatenate([s.T, np.ones((1, N_SH), np.float32)], axis=0)
        tgtT4 = np.concatenate([t.T, np.ones((1, M), np.float32)], axis=0)
        im = {
            "lhsT": np.ascontiguousarray(lhsT),
            "rhs": np.ascontiguousarray(rhs),
            "srcT4": np.ascontiguousarray(srcT4.astype(np.float32)),
            "tgtT4": np.ascontiguousarray(tgtT4.astype(np.float32)),
            "w1b": w1b,
            "w2": np.ascontiguousarray(np.concatenate([W2, W2], axis=0)),
            "b2t": b2t,
            "ltc": ltc,
        }
        if not trivial_ln:
            im["lng"] = np.tile(ln_g[None, :], (128, 1)).astype(np.float32)
            im["lnb"] = np.tile(ln_b[None, :], (128, 1)).astype(np.float32)
        in_maps.append(im)

    global _last_in_maps
    _last_in_maps = in_maps
    res = run_bass_kernel_spmd(nc, in_maps, core_ids=list(range(8)))
    out = np.zeros((B, N, 3), np.float32)
    for c in range(8):
        b = c // 2
        h = c % 2
        out[b, h * N_SH:(h + 1) * N_SH] = res.results[c]["out"]
    return out


# revision 15
# speedup vs baseline: 1.3302x; 1.0149x over previous
"""Trainium2 Bass kernel for nn_CrossAttentionGenerator.

Pipeline (per core; 8 cores = 4 batches x 2 halves of N):
  - MLP features for the core's source half (4096 rows) and the full target
    (8192 rows) of its batch: Linear(3->64) -> LayerNorm -> ReLU -> Linear(64->64).
    Target features (WITHOUT b2 -- a per-k-constant logit shift, softmax
    invariant) + coords go to a DRAM table for gathering.
  - Distance phase per 128-row source tile: v = 2 s.t - |t|^2 (|s|^2 dropped:
    per-row constant, ranking-invariant) via split-bf16 matmul (K=21) into
    PSUM, evacuated to SBUF by the scalar engine.
  - Top-8 per row: nc.vector.max + nc.vector.max_index into persistent slabs,
    so the vector-bound selection stream never blocks on the attention chain.
  - Attention (1-tile lag behind selection): gather [feat|coords] rows via
    per-partition indirect DMA, logits = srcF . K / temp, softmax, attended.

Self-contained: hardcodes B=4, N=M=8192, F=64, K=8 and does all sharding
host-side inside kernel().
"""

import numpy as np

import concourse.bacc as bacc
import concourse.bass as bass
import concourse.tile as tile
import concourse.mybir as mybir
from concourse.bass_utils import run_bass_kernel_spmd
from concourse.masks import make_identity

import ml_dtypes

bf16 = ml_dtypes.bfloat16
f32 = mybir.dt.float32
bf16_t = mybir.dt.bfloat16
u16 = mybir.dt.uint16
u32 = mybir.dt.uint32

B, N, M, F = 4, 8192, 8192, 64
K_NN = 8
N_SH = N // 2            # rows per core
TILES = N_SH // 128      # 32 source tiles
MTILES = M // 128        # 64 target tiles
K_SPLIT = 21             # split-bf16 distance matmul contraction size
TBL_W = 68               # feat(64) + coords(3) + pad(1)
LN_EPS = 1e-5

_CACHE = {}


def _split3(x):
    """fp32 -> three bf16 planes (h+m+l reconstructs x to ~2^-24 rel)."""
    h = x.astype(bf16).astype(np.float32)
    r = (x - h).astype(np.float32)
    m = r.astype(bf16).astype(np.float32)
    l = (r - m).astype(np.float32).astype(bf16).astype(np.float32)
    return h, m, l


def _build_dist_strips(s, t):
    """lhsT (21, n) and rhs (21, m) bf16 strips for v = 2 s.t - |t|^2.

    |s|^2 is omitted: it is constant along each distance row, so the top-k
    ranking and the matched values' indices are unchanged.  Per coordinate
    the six O(>=2^-24) product terms are kept: hh hm mh mm hl lh; the |t|^2
    column constant is subtracted via three (-1, t2*) rows.
    """
    n = s.shape[0]; m = t.shape[0]
    sh, sm, sl = _split3(2.0 * s)
    th, tm_, tl = _split3(t)
    tsq = ((t[:, 0] * t[:, 0]).astype(np.float32)
           + (t[:, 1] * t[:, 1]).astype(np.float32))
    tsq = (tsq.astype(np.float32) + (t[:, 2] * t[:, 2]).astype(np.float32)).astype(np.float32)
    t2h, t2m, t2l = _split3(tsq)
    ones_n = np.ones((n,), np.float32)

    lhs_rows = []
    rhs_rows = []
    for c in range(3):
        pairs = [(sh[:, c], th[:, c]), (sh[:, c], tm_[:, c]), (sm[:, c], th[:, c]),
                 (sm[:, c], tm_[:, c]), (sh[:, c], tl[:, c]), (sl[:, c], th[:, c])]
        for a, b in pairs:
            lhs_rows.append(a)
            rhs_rows.append(b)
    for r in (t2h, t2m, t2l):
        lhs_rows.append(-ones_n)
        rhs_rows.append(r)
    lhsT = np.stack(lhs_rows).astype(bf16)   # (21, n)
    rhs = np.stack(rhs_rows).astype(bf16)    # (21, m)
    return lhsT, rhs


def _build_program(trivial_ln):
    nc = bacc.Bacc("TRN2", target_bir_lowering=False, num_devices=8)

    # ---- I/O -------------------------------------------------------------
    lhsT_d = nc.dram_tensor("lhsT", [K_SPLIT, N_SH], bf16_t, kind="ExternalInput")
    rhs_d = nc.dram_tensor("rhs", [K_SPLIT, M], bf16_t, kind="ExternalInput")
    srcT4_d = nc.dram_tensor("srcT4", [4, N_SH], f32, kind="ExternalInput")
    tgtT4_d = nc.dram_tensor("tgtT4", [4, M], f32, kind="ExternalInput")
    w1b_d = nc.dram_tensor("w1b", [4, F], f32, kind="ExternalInput")
    w2_d = nc.dram_tensor("w2", [2 * F, F], f32, kind="ExternalInput")
    b2t_d = nc.dram_tensor("b2t", [128, F], f32, kind="ExternalInput")
    ltc_d = nc.dram_tensor("ltc", [128, 1], f32, kind="ExternalInput")
    if not trivial_ln:
        lng_d = nc.dram_tensor("lng", [128, F], f32, kind="ExternalInput")
        lnb_d = nc.dram_tensor("lnb", [128, F], f32, kind="ExternalInput")
    out_d = nc.dram_tensor("out", [N_SH, 3], f32, kind="ExternalOutput")

    with tile.TileContext(nc) as tc:
        import contextlib
        ctx = contextlib.ExitStack()
        with ctx:
            const = ctx.enter_context(tc.tile_pool(name="const", bufs=1))
            mlp_sb = ctx.enter_context(tc.tile_pool(name="mlp_sb", bufs=4))
            mlp_sm = ctx.enter_context(tc.tile_pool(name="mlp_sm", bufs=8))
            mlp_sq = ctx.enter_context(tc.tile_pool(name="mlp_sq", bufs=2))
            mlp_ps = ctx.enter_context(tc.tile_pool(name="mlp_ps", bufs=2, space="PSUM"))
            dist_ps = ctx.enter_context(tc.tile_pool(name="dist_ps", bufs=2, space="PSUM"))
            row_sb = ctx.enter_context(tc.tile_pool(name="row_sb", bufs=2))
            att_sb = ctx.enter_context(tc.tile_pool(name="att_sb", bufs=3))
            dram = ctx.enter_context(tc.tile_pool(name="dram", bufs=1, space="DRAM"))

            # ---- constants / strips -------------------------------------
            lhsT = const.tile([K_SPLIT, N_SH], bf16_t)
            nc.sync.dma_start(lhsT[:], lhsT_d[:])
            rhs = const.tile([K_SPLIT, M], bf16_t)
            nc.sync.dma_start(rhs[:], rhs_d[:])
            srcT4 = const.tile([4, N_SH], f32)
            nc.sync.dma_start(srcT4[:], srcT4_d[:])
            tgtT4 = const.tile([4, M], f32)
            nc.sync.dma_start(tgtT4[:], tgtT4_d[:])
            w1b = const.tile([4, F], f32)
            nc.sync.dma_start(w1b[:], w1b_d[:])
            w2 = const.tile([2 * F, F], f32)   # W2 duplicated at partitions 0 and 64
            nc.sync.dma_start(w2[:], w2_d[:])
            b2t = const.tile([128, F], f32)
            nc.sync.dma_start(b2t[:], b2t_d[:])
            ltc = const.tile([128, 1], f32)
            nc.sync.dma_start(ltc[:], ltc_d[:])
            if not trivial_ln:
                lng = const.tile([128, F], f32)
                nc.sync.dma_start(lng[:], lng_d[:])
                lnb = const.tile([128, F], f32)
                nc.sync.dma_start(lnb[:], lnb_d[:])

            ident = const.tile([128, 128], f32)
            make_identity(nc, ident[:])

            # inv temperature column: 0.125 * exp(-log_temp)
            invt = const.tile([128, 1], f32)
            nc.scalar.activation(invt[:], ltc[:], mybir.ActivationFunctionType.Exp,
                                 scale=-1.0)
            nc.vector.tensor_scalar_mul(invt[:], invt[:], 1.0 / 8.0)

            srcF = const.tile([128, TILES * F], f32)       # source features (+b2)
            outacc = const.tile([128, TILES * 3], f32)     # attended accumulator
            v8slab = const.tile([128, TILES * 8], f32)     # top-8 values per tile
            m8slab = const.tile([128, TILES * 8], u32)     # top-8 indices per tile

            # feature table in DRAM: [feat(64) | coords(3) | pad]
            ftable = dram.tile([M, TBL_W], f32)
            # coords columns from tgtT4 rows 0:3 (one strided DMA)
            nc.sync.dma_start(
                ftable[:, 64:67].rearrange("m c -> c m"), tgtT4[0:3, :])

            # ---- MLP: batched-stats structure ---------------------------
            # Phase A: mm1 for every tile -> h slab (PSUM freed immediately).
            # Phase B: segmented bn_stats over 8 tiles per call + slab-wide
            #          istd/nmu chain  (a handful of vector ops total, so the
            #          MLP never competes with the 8.7us selection scans for
            #          vector slots).
            # Phase C: per pair: z=Relu(h*istd+nmu), one 128x128 transpose,
            #          mm2, feature writeback.
            NT = MTILES + TILES                         # 96 tiles total
            hslab = const.tile([128, NT * F], f32)      # 24KB/partition
            sums = const.tile([128, NT], f32)           # per-tile sum(h)
            sumsq = const.tile([128, NT], f32)          # per-tile sum(h^2)
            istds = const.tile([128, NT], f32)
            nmus = const.tile([128, NT], f32)

            def mlp_A(idx, xap):
                h_ps = mlp_ps.tile([128, F], f32, tag="mm")
                nc.tensor.matmul(h_ps[:], lhsT=xap, rhs=w1b[:],
                                 start=True, stop=True)
                nc.scalar.copy(hslab[:, idx * F:(idx + 1) * F], h_ps[:])

            def mlp_stats(g):
                # segmented sum / sum-of-squares over 8 tiles at once
                sl = hslab[:, g * 8 * F:(g + 1) * 8 * F].rearrange(
                    "p (t f) -> p t f", f=F)
                sq = mlp_sq.tile([128, 8, F], f32, tag="sq")
                nc.scalar.activation(sq[:], sl,
                                     mybir.ActivationFunctionType.Square)
                nc.vector.tensor_reduce(
                    out=sums[:, g * 8:(g + 1) * 8], in_=sl,
                    op=mybir.AluOpType.add, axis=mybir.AxisListType.X)
                nc.vector.tensor_reduce(
                    out=sumsq[:, g * 8:(g + 1) * 8], in_=sq[:],
                    op=mybir.AluOpType.add, axis=mybir.AxisListType.X)

            def mlp_chain(lo, nk):
                # istd = 1/sqrt(E[h^2] - mu^2 + eps); nmu = -mu*istd
                mu = mlp_sm.tile([128, nk], f32, tag="mu%d" % lo)
                nc.vector.tensor_scalar_mul(mu[:], sums[:, lo:lo + nk], 1.0 / F)
                var = mlp_sm.tile([128, nk], f32, tag="var%d" % lo)
                nc.vector.scalar_tensor_tensor(
                    out=var[:], in0=mu[:], scalar=-1.0, in1=mu[:],
                    op0=mybir.AluOpType.mult, op1=mybir.AluOpType.mult)
                nc.vector.scalar_tensor_tensor(
                    out=var[:], in0=sumsq[:, lo:lo + nk], scalar=1.0 / F,
                    in1=var[:],
                    op0=mybir.AluOpType.mult, op1=mybir.AluOpType.add)
                nc.vector.tensor_scalar_add(var[:], var[:], LN_EPS)
                nc.vector.reciprocal(var[:], var[:])
                nc.scalar.activation(istds[:, lo:lo + nk], var[:],
                                     mybir.ActivationFunctionType.Sqrt)
                nc.vector.scalar_tensor_tensor(
                    out=nmus[:, lo:lo + nk], in0=mu[:],
                    scalar=-1.0, in1=istds[:, lo:lo + nk],
                    op0=mybir.AluOpType.mult, op1=mybir.AluOpType.mult)

            def mlp_C(pair, dst_kind):
                # global tile indices a, a+1
                a = 2 * pair
                z2 = mlp_sb.tile([128, 2 * F], f32, tag="z2")
                for j in range(2):
                    idx = a + j
                    z = z2[:, j * F:(j + 1) * F]
                    h = hslab[:, idx * F:(idx + 1) * F]
                    if trivial_ln:
                        nc.scalar.activation(z, h,
                                             mybir.ActivationFunctionType.Relu,
                                             bias=nmus[:, idx:idx + 1],
                                             scale=istds[:, idx:idx + 1])
                    else:
                        zn = mlp_sb.tile([128, F], f32, tag="zn")
                        nc.scalar.activation(zn[:], h,
                                             mybir.ActivationFunctionType.Identity,
                                             bias=nmus[:, idx:idx + 1],
                                             scale=istds[:, idx:idx + 1])
                        nc.vector.scalar_tensor_tensor(
                            out=zn[:], in0=zn[:], scalar=1.0, in1=lng[:],
                            op0=mybir.AluOpType.mult, op1=mybir.AluOpType.mult)
                        nc.vector.scalar_tensor_tensor(
                            out=zn[:], in0=zn[:], scalar=0.0, in1=lnb[:],
                            op0=mybir.AluOpType.add, op1=mybir.AluOpType.add)
                        nc.vector.tensor_relu(z, zn[:])
                zt_ps = mlp_ps.tile([128, 128], f32, tag="tr")
                nc.tensor.transpose(out=zt_ps[:], in_=z2[:], identity=ident[:])
                hT2 = mlp_sb.tile([128, 128], f32, tag="hT2")
                nc.scalar.copy(hT2[:], zt_ps[:])
                for j in range(2):
                    f_ps = mlp_ps.tile([128, F], f32, tag="mm")
                    nc.tensor.matmul(f_ps[:], lhsT=hT2[j * F:(j + 1) * F, :],
                                     rhs=w2[j * F:(j + 1) * F, :],
                                     start=True, stop=True)
                    if dst_kind == "tgt":
                        t_idx = a + j
                        feat = mlp_sb.tile([128, F], f32, tag="feat")
                        nc.scalar.copy(feat[:], f_ps[:])
                        nc.sync.dma_start(
                            ftable[t_idx * 128:(t_idx + 1) * 128, 0:F], feat[:])
                    else:
                        t_idx = a + j - MTILES
                        nc.vector.tensor_tensor(
                            out=srcF[:, t_idx * F:(t_idx + 1) * F],
                            in0=f_ps[:], in1=b2t[:], op=mybir.AluOpType.add)

            # target tiles first so ftable completes early
            for i in range(MTILES):
                mlp_A(i, tgtT4[:, i * 128:(i + 1) * 128])
            for g in range(MTILES // 8):
                mlp_stats(g)
            mlp_chain(0, MTILES)
            for p in range(MTILES // 2):
                mlp_C(p, "tgt")
            for i in range(TILES):
                mlp_A(MTILES + i, srcT4[:, i * 128:(i + 1) * 128])
            for g in range(MTILES // 8, NT // 8):
                mlp_stats(g)
            mlp_chain(MTILES, TILES)
            for p in range(MTILES // 2, NT // 2):
                mlp_C(p, "src")

            # ---- selection per tile (vector-bound stream) ---------------
            NGRP = 8          # PSUM evacuation groups per tile
            GW = M // NGRP    # 1024 columns per group

            def select_tile(t):
                lhs_t = lhsT[:, t * 128:(t + 1) * 128]
                row = row_sb.tile([128, M], f32, tag="row")
                # row production must never queue behind MLP ops: the vector
                # scans are the critical path and they feed on these rows
                with tc.high_priority():
                    for g in range(NGRP):
                        ps = dist_ps.tile([128, GW], f32, tag="d")
                        for h in range(GW // 512):
                            c0 = g * GW + h * 512
                            nc.tensor.matmul(ps[:, h * 512:(h + 1) * 512],
                                             lhsT=lhs_t,
                                             rhs=rhs[:, c0:c0 + 512],
                                             start=True, stop=True)
                        nc.scalar.copy(row[:, g * GW:(g + 1) * GW], ps[:])

                # exact top-8 (maximum v = closest) into persistent slabs
                v8 = v8slab[:, t * 8:(t + 1) * 8]
                nc.vector.max(out=v8, in_=row[:])
                nc.vector.max_index(out=m8slab[:, t * 8:(t + 1) * 8],
                                    in_max=v8, in_values=row[:])

                # gathers queued on gpsimd right away (ftable ready after MLP)
                gath = att_sb.tile([128, K_NN, TBL_W], f32, tag="gath")
                for k in range(K_NN):
                    nc.gpsimd.indirect_dma_start(
                        out=gath[:, k, :], out_offset=None,
                        in_=ftable[:],
                        in_offset=bass.IndirectOffsetOnAxis(
                            ap=m8slab[:, t * 8 + k:t * 8 + k + 1], axis=0))
                return gath

            def attend_tile(t, gath):
                # logits_k = srcF . K_k
                logits = att_sb.tile([128, K_NN], f32, tag="logits")
                scr = att_sb.tile([128, F], f32, tag="scr")
                sf = srcF[:, t * F:(t + 1) * F]
                for k in range(K_NN):
                    nc.vector.scalar_tensor_tensor(
                        out=scr[:], in0=gath[:, k, 0:F], scalar=1.0, in1=sf,
                        op0=mybir.AluOpType.mult, op1=mybir.AluOpType.mult,
                        accum_out=logits[:, k:k + 1])
                # softmax over k with temperature
                mx = att_sb.tile([128, 1], f32, tag="mx")
                nc.vector.tensor_reduce(out=mx[:], in_=logits[:],
                                        op=mybir.AluOpType.max,
                                        axis=mybir.AxisListType.X)
                bcol = att_sb.tile([128, 1], f32, tag="bcol")
                nc.vector.scalar_tensor_tensor(
                    out=bcol[:], in0=mx[:], scalar=-1.0, in1=invt[:],
                    op0=mybir.AluOpType.mult, op1=mybir.AluOpType.mult)
                att = att_sb.tile([128, K_NN], f32, tag="att")
                nc.scalar.activation(att[:], logits[:],
                                     mybir.ActivationFunctionType.Exp,
                                     bias=bcol[:], scale=invt[:])
                ssum = att_sb.tile([128, 1], f32, tag="ssum")
                nc.vector.tensor_reduce(out=ssum[:], in_=att[:],
                                        op=mybir.AluOpType.add,
                                        axis=mybir.AxisListType.X)
                rs = att_sb.tile([128, 1], f32, tag="rs")
                nc.vector.reciprocal(rs[:], ssum[:])
                nc.vector.tensor_scalar_mul(att[:], att[:], rs[:])
                # attended = sum_k att * V  (V = gathered coords)
                prod = att_sb.tile([128, K_NN, 3], f32, tag="prod")
                nc.vector.scalar_tensor_tensor(
                    out=prod[:], in0=gath[:, :, F:F + 3], scalar=1.0,
                    in1=att[:].to_broadcast([128, K_NN, 3]),
                    op0=mybir.AluOpType.mult, op1=mybir.AluOpType.mult)
                nc.vector.tensor_reduce(
                    out=outacc[:, t * 3:(t + 1) * 3],
                    in_=prod[:].rearrange("p k c -> p c k"),
                    op=mybir.AluOpType.add, axis=mybir.AxisListType.X)

            pend = None
            for t in range(TILES):
                gath = select_tile(t)
                if pend is not None:
                    attend_tile(t - 1, pend)
                pend = gath
            attend_tile(TILES - 1, pend)

            # ---- write output -------------------------------------------
            nc.sync.dma_start(
                out_d[:].rearrange("(t p) c -> p t c", p=128),
                outacc[:].rearrange("p (t c) -> p t c", c=3))

    nc.compile()
    return nc


def _get_program(trivial_ln):
    key = ("prog", trivial_ln)
    if key not in _CACHE:
        _CACHE[key] = _build_program(trivial_ln)
    return _CACHE[key]


def kernel(source, target, W1, b1, ln_g, ln_b, W2, b2, log_temp):
    source = np.ascontiguousarray(np.asarray(source, dtype=np.float32))
    target = np.ascontiguousarray(np.asarray(target, dtype=np.float32))
    W1 = np.asarray(W1, np.float32); b1 = np.asarray(b1, np.float32)
    ln_g = np.asarray(ln_g, np.float32); ln_b = np.asarray(ln_b, np.float32)
    W2 = np.asarray(W2, np.float32); b2 = np.asarray(b2, np.float32)
    log_temp = np.asarray(log_temp, np.float32)

    trivial_ln = bool(np.all(ln_g == 1.0) and np.all(ln_b == 0.0))
    nc = _get_program(trivial_ln)

    w1b = np.concatenate([W1, b1[None, :]], axis=0).astype(np.float32)       # (4, 64)
    b2t = np.tile(b2[None, :], (128, 1)).astype(np.float32)
    ltc = np.full((128, 1), float(log_temp[0]), np.float32)

    in_maps = []
    for c in range(8):
        b = c // 2
        h = c % 2
        s = source[b, h * N_SH:(h + 1) * N_SH]      # (4096, 3)
        t = target[b]                                # (8192, 3)
        lhsT, rhs = _build_dist_strips(s, t)
        srcT4 = np.concatenate([s.T, np.ones((1, N_SH), np.float32)], axis=0)
        tgtT4 = np.concatenate([t.T, np.ones((1, M), np.float32)], axis=0)
        im = {
            "lhsT": np.ascontiguousarray(lhsT),
            "rhs": np.ascontiguousarray(rhs),
            "srcT4": np.ascontiguousarray(srcT4.astype(np.float32)),
            "tgtT4": np.ascontiguousarray(tgtT4.astype(np.float32)),
            "w1b": w1b,
            "w2": np.ascontiguousarray(np.concatenate([W2, W2], axis=0)),
            "b2t": b2t,
            "ltc": ltc,
        }
        if not trivial_ln:
            im["lng"] = np.tile(ln_g[None, :], (128, 1)).astype(np.float32)
            im["lnb"] = np.tile(ln_b[None, :], (128, 1)).astype(np.float32)
        in_maps.append(im)

    global _last_in_maps
    _last_in_maps = in_maps
    res = run_bass_kernel_spmd(nc, in_maps, core_ids=list(range(8)))
    out = np.zeros((B, N, 3), np.float32)
    for c in range(8):
        b = c // 2
        h = c % 2
        out[b, h * N_SH:(h + 1) * N_SH] = res.results[c]["out"]
    return out
